# revision 1
# baseline (speedup 1.0000x reference)
"""Trainium2 Bass kernel for single-head base attention.

Problem: x [4, 2048, 1024] fp32; Wq/Wk/Wv [1024, 1024] (torch [out, in]).
  Q = x @ Wq.T ; K = x @ Wk.T ; V = x @ Wv.T
  out = softmax(Q K^T / 32) V

Sharding: 8 cores = 4 batches x 2 query-halves. Each core computes K/V for
its batch's full 2048-seq and Q for its 1024-query half; outputs are
disjoint [1024, 1024] slices, so no collectives.

Per-core schedule (all matmuls float32r = tf32-rate, fp32 accumulate):
  phase Q: QT[e,q] = WqT.T @ xqT     -> spilled to DRAM scratch
  phase V: V[k,e]  = xT.T @ WvT      -> resident SBUF (8MB)
  phase K: KT[e,k] = WkT.T @ xT      -> resident SBUF (8MB)
  attention per 128-query tile:
      S[q,k] = QT_sl.T @ KT          (PSUM, 4 banks)
      attU = exp(S/32) (ACT, accum_out = row sums; no max-subtraction
             needed: |S/32| <= ~6 so exp is well within fp32 range)
      attT = PE-transpose(attU) 128x128 blocks
      O[q,e] = attT.T @ V            (PSUM accumulate over k)
      out = O * (1/rowsum)           -> DRAM
"""

import os
import sys
from contextlib import ExitStack

import numpy as np

for _p in ("/opt/trn_rl_repo", "/root/.axon_site/_ro/trn_rl_repo"):
    if os.path.isdir(_p) and _p not in sys.path:
        sys.path.append(_p)

import concourse.bass as bass
import concourse.mybir as mybir
from concourse import bacc, tile
from concourse.bass_utils import run_bass_kernel_spmd

F32 = mybir.dt.float32
F32R = mybir.dt.float32r  # tf32-rate matmul dtype, fp32 storage bits

B, SEQ, D = 4, 2048, 1024
QL = SEQ // 2          # queries per core
N_CORES = 8
DT = D // 128          # 8 d-tiles (contraction)
ET = D // 128          # 8 e-tiles (hidden out)
KT = SEQ // 128        # 16 k-tiles
QT = QL // 128         # 8 q-tiles per core
XB = 256               # xT streaming col-block width
NXB = SEQ // XB        # 8 blocks
AF = mybir.ActivationFunctionType


def _copy(nc, i, dst, src):
    # alternate PSUM->SBUF copies between DVE and ACT to balance engines
    if i % 2 == 0:
        nc.vector.tensor_copy(dst, src)
    else:
        nc.scalar.copy(dst, src)


def build():
    nc = bacc.Bacc(
        "TRN2", target_bir_lowering=False, debug=False, num_devices=N_CORES
    )

    xT = nc.declare_dram_parameter("xT", [D, SEQ], F32R, isOutput=False)
    xqT = nc.declare_dram_parameter("xqT", [D, QL], F32R, isOutput=False)
    WqT = nc.declare_dram_parameter("WqT", [D, D], F32R, isOutput=False)
    WkT = nc.declare_dram_parameter("WkT", [D, D], F32R, isOutput=False)
    WvT = nc.declare_dram_parameter("WvT", [D, D], F32R, isOutput=False)
    idn = nc.declare_dram_parameter("idn", [128, 128], F32, isOutput=False)
    out = nc.declare_dram_parameter("out", [QL, D], F32, isOutput=True)

    xT_r = xT.rearrange("(dt p) k -> p dt k", p=128)
    xqT_r = xqT.rearrange("(dt p) q -> p dt q", p=128)
    Wq_r = WqT.rearrange("(dt p) e -> p dt e", p=128)
    Wk_r = WkT.rearrange("(dt p) e -> p dt e", p=128)
    Wv_r = WvT.rearrange("(dt p) e -> p dt e", p=128)
    out_r = out.rearrange("(qt p) e -> qt p e", p=128)

    with ExitStack() as top:
        tc = top.enter_context(tile.TileContext(nc))

        const_pool = top.enter_context(tc.tile_pool(name="const", bufs=1))
        res_pool = top.enter_context(tc.tile_pool(name="res", bufs=1))
        dram_pool = top.enter_context(
            tc.tile_pool(name="dram", bufs=1, space="DRAM")
        )

        ident = const_pool.tile([128, 128], F32)
        nc.sync.dma_start(ident[:], idn[:])

        v_sb = res_pool.tile([128, KT, D], F32R, tag="v_sb")
        kt_sb = res_pool.tile([128, ET, SEQ], F32R, tag="kt_sb")
        qts = dram_pool.tile([ET, 128, QL], F32R, tag="qts")

        # ---------------- phase Q: QT[e,q], spilled to DRAM ----------------
        with (
            tc.tile_pool(name="phq_w", bufs=1) as phq_w,
            tc.tile_pool(name="phq_s", bufs=4) as phq_s,
            tc.tile_pool(name="psq", bufs=4, space="PSUM") as psq,
        ):
            wq = phq_w.tile([128, DT, D], F32R, tag="wq")
            nc.sync.dma_start(wq[:], Wq_r[:])
            xq = phq_w.tile([128, DT, QL], F32R, tag="xq")
            nc.sync.dma_start(xq[:], xqT_r[:])
            ci = 0
            for qc in range(QL // 512):
                for et in range(ET):
                    ps = psq.tile([128, 512], F32, tag="ps")
                    for d in range(DT):
                        nc.tensor.matmul(
                            ps[:],
                            wq[:, d, et * 128 : (et + 1) * 128],
                            xq[:, d, qc * 512 : (qc + 1) * 512],
                            start=(d == 0),
                            stop=(d == DT - 1),
                        )
                    piece = phq_s.tile([128, 512], F32R, tag="piece")
                    _copy(nc, ci, piece[:], ps[:])
                    ci += 1
                    nc.sync.dma_start(
                        qts[et, :, qc * 512 : (qc + 1) * 512], piece[:]
                    )

        # ---------------- phase V: V[k,e] resident ----------------
        with (
            tc.tile_pool(name="phv_w", bufs=1) as phv_w,
            tc.tile_pool(name="phv_x", bufs=2) as phv_x,
            tc.tile_pool(name="psv", bufs=4, space="PSUM") as psv,
        ):
            wv = phv_w.tile([128, DT, D], F32R, tag="wv")
            nc.sync.dma_start(wv[:], Wv_r[:])
            ci = 0
            for j in range(NXB):
                xtb = phv_x.tile([128, DT, XB], F32R, tag="xtb")
                nc.sync.dma_start(xtb[:], xT_r[:, :, j * XB : (j + 1) * XB])
                for k2 in range(XB // 128):
                    kt = j * (XB // 128) + k2
                    for ec in range(D // 512):
                        ps = psv.tile([128, 512], F32, tag="ps")
                        for d in range(DT):
                            nc.tensor.matmul(
                                ps[:],
                                xtb[:, d, k2 * 128 : (k2 + 1) * 128],
                                wv[:, d, ec * 512 : (ec + 1) * 512],
                                start=(d == 0),
                                stop=(d == DT - 1),
                            )
                        _copy(nc, ci, v_sb[:, kt, ec * 512 : (ec + 1) * 512], ps[:])
                        ci += 1

        # ---------------- phase K: KT[e,k] resident ----------------
        with (
            tc.tile_pool(name="phk_w", bufs=1) as phk_w,
            tc.tile_pool(name="phk_x", bufs=2) as phk_x,
            tc.tile_pool(name="psk", bufs=4, space="PSUM") as psk,
        ):
            wk = phk_w.tile([128, DT, D], F32R, tag="wk")
            nc.sync.dma_start(wk[:], Wk_r[:])
            ci = 0
            for j in range(NXB):
                xtb = phk_x.tile([128, DT, XB], F32R, tag="xtb")
                nc.sync.dma_start(xtb[:], xT_r[:, :, j * XB : (j + 1) * XB])
                for et in range(ET):
                    ps = psk.tile([128, XB], F32, tag="ps")
                    for d in range(DT):
                        nc.tensor.matmul(
                            ps[:],
                            wk[:, d, et * 128 : (et + 1) * 128],
                            xtb[:, d, :],
                            start=(d == 0),
                            stop=(d == DT - 1),
                        )
                    _copy(nc, ci, kt_sb[:, et, j * XB : (j + 1) * XB], ps[:])
                    ci += 1

        # ---------------- attention ----------------
        with (
            tc.tile_pool(name="qsl_p", bufs=2) as qsl_p,
            tc.tile_pool(name="attu_p", bufs=2) as attu_p,
            tc.tile_pool(name="attt_p", bufs=4) as attt_p,
            tc.tile_pool(name="osb_p", bufs=2) as osb_p,
            tc.tile_pool(name="vec_p", bufs=4) as vec_p,
            tc.tile_pool(name="pss", bufs=1, space="PSUM") as pss,
            tc.tile_pool(name="pst", bufs=2, space="PSUM") as pst,
            tc.tile_pool(name="pso", bufs=2, space="PSUM") as pso,
        ):
            qts_r = qts[:].rearrange("et p q -> p et q")
            for qt in range(QT):
                qsl = qsl_p.tile([128, ET, 128], F32R, tag="qsl")
                nc.sync.dma_start(
                    qsl[:], qts_r[:, :, qt * 128 : (qt + 1) * 128]
                )
                S = pss.tile([128, SEQ], F32, tag="S")
                for et in range(ET):
                    for kc in range(SEQ // 512):
                        nc.tensor.matmul(
                            S[:, kc * 512 : (kc + 1) * 512],
                            qsl[:, et, :],
                            kt_sb[:, et, kc * 512 : (kc + 1) * 512],
                            start=(et == 0),
                            stop=(et == ET - 1),
                        )
                attu = attu_p.tile([128, SEQ], F32, tag="attu")
                rs = vec_p.tile([128, 1], F32, tag="rs")
                nc.scalar.activation(
                    attu[:], S[:], AF.Exp, scale=1.0 / 32.0, accum_out=rs[:]
                )
                r = vec_p.tile([128, 1], F32, tag="r")
                nc.vector.reciprocal(r[:], rs[:])

                po0 = pso.tile([128, 512], F32, tag="po")
                po1 = pso.tile([128, 512], F32, tag="po")
                pos = (po0, po1)
                for kt in range(KT):
                    tp = pst.tile([128, 128], F32, tag="tp")
                    nc.tensor.transpose(
                        tp[:], attu[:, kt * 128 : (kt + 1) * 128], ident[:]
                    )
                    at = attt_p.tile([128, 128], F32R, tag="at")
                    _copy(nc, kt, at[:], tp[:])
                    for ec in range(2):
                        nc.tensor.matmul(
                            pos[ec][:],
                            at[:],
                            v_sb[:, kt, ec * 512 : (ec + 1) * 512],
                            start=(kt == 0),
                            stop=(kt == KT - 1),
                        )
                osb = osb_p.tile([128, D], F32, tag="osb")
                for ec in range(2):
                    nc.vector.tensor_scalar_mul(
                        osb[:, ec * 512 : (ec + 1) * 512], pos[ec][:], r[:]
                    )
                nc.sync.dma_start(out_r[qt], osb[:])

    nc.compile()
    return nc


_CACHE: dict = {}


def _get_nc():
    if "nc" not in _CACHE:
        _CACHE["nc"] = build()
    return _CACHE["nc"]


def _make_in_maps(x, Wq, Wk, Wv):
    x = np.asarray(x, dtype=np.float32)
    wqT = np.ascontiguousarray(np.asarray(Wq, dtype=np.float32).T)
    wkT = np.ascontiguousarray(np.asarray(Wk, dtype=np.float32).T)
    wvT = np.ascontiguousarray(np.asarray(Wv, dtype=np.float32).T)
    eye = np.eye(128, dtype=np.float32)
    in_maps = []
    for c in range(N_CORES):
        b, h = divmod(c, 2)
        xb = np.ascontiguousarray(x[b].T)
        xq = np.ascontiguousarray(x[b, h * QL : (h + 1) * QL].T)
        in_maps.append(
            {
                "xT": xb,
                "xqT": xq,
                "WqT": wqT,
                "WkT": wkT,
                "WvT": wvT,
                "idn": eye,
            }
        )
    return in_maps


def _assemble(results):
    out = np.empty((B, SEQ, D), dtype=np.float32)
    for c in range(N_CORES):
        b, h = divmod(c, 2)
        out[b, h * QL : (h + 1) * QL] = results[c]["out"]
    return out


def run_traced(x, Wq, Wk, Wv, **kw):
    """Run and return (output, BassKernelResults) - used by test.py."""
    nc = _get_nc()
    res = run_bass_kernel_spmd(
        nc, _make_in_maps(x, Wq, Wk, Wv), list(range(N_CORES)), **kw
    )
    return _assemble(res.results), res


def kernel(x, Wq, Wk, Wv):
    out, _ = run_traced(x, Wq, Wk, Wv)
    return out


# revision 6
# speedup vs baseline: 72.3450x; 72.3450x over previous
"""Trainium2 Bass kernel for single-head base attention.

Problem: x [4, 2048, 1024] fp32; Wq/Wk/Wv [1024, 1024] (torch [out, in]).
  Q = x @ Wq.T ; K = x @ Wk.T ; V = x @ Wv.T
  out = softmax(Q K^T / 32) V

Sharding: 8 cores = 4 batches x 2 query-halves. Each core computes K/V for
its batch's full 2048-seq and Q for its 1024-query half; outputs are
disjoint [1024, 1024] slices, so no collectives.

Per-core schedule (all matmuls float32r = tf32-rate, fp32 accumulate):
  phase Q: QT[e,q] = WqT.T @ xqT     -> spilled to DRAM scratch
  phase V: V[k,e]  = xT.T @ WvT      -> resident SBUF (8MB)
  phase K: KT[e,k] = WkT.T @ xT      -> resident SBUF (8MB)
  attention per 128-query tile:
      S[q,k] = QT_sl.T @ KT          (PSUM, 4 banks)
      attU = exp(S/32) (ACT, accum_out = row sums; no max-subtraction
             needed: |S/32| <= ~6 so exp is well within fp32 range)
      attT = PE-transpose(attU) 128x128 blocks
      O[q,e] = attT.T @ V            (PSUM accumulate over k)
      out = O * (1/rowsum)           -> DRAM
"""

import os
import sys
from contextlib import ExitStack

import numpy as np

for _p in ("/opt/trn_rl_repo", "/root/.axon_site/_ro/trn_rl_repo"):
    if os.path.isdir(_p) and _p not in sys.path:
        sys.path.append(_p)

import concourse.bass as bass
import concourse.mybir as mybir
from concourse import bacc, tile
from concourse.bass_utils import run_bass_kernel_spmd

F32 = mybir.dt.float32
F32R = mybir.dt.float32r  # tf32-rate matmul dtype, fp32 storage bits

B, SEQ, D = 4, 2048, 1024
QL = SEQ // 2          # queries per core
N_CORES = 8
DT = D // 128          # 8 d-tiles (contraction)
ET = D // 128          # 8 e-tiles (hidden out)
KT = SEQ // 128        # 16 k-tiles
QT = QL // 128         # 8 q-tiles per core
XB = 256               # xT streaming col-block width
NXB = SEQ // XB        # 8 blocks
AF = mybir.ActivationFunctionType


def _copy(nc, i, dst, src):
    # alternate PSUM->SBUF copies between DVE and ACT to balance engines
    if i % 2 == 0:
        nc.vector.tensor_copy(dst, src)
    else:
        nc.scalar.copy(dst, src)


def build():
    nc = bacc.Bacc(
        "TRN2", target_bir_lowering=False, debug=False, num_devices=N_CORES
    )

    xT = nc.declare_dram_parameter("xT", [D, SEQ], F32R, isOutput=False)
    xqT = nc.declare_dram_parameter("xqT", [D, QL], F32R, isOutput=False)
    WqT = nc.declare_dram_parameter("WqT", [D, D], F32R, isOutput=False)
    WkT = nc.declare_dram_parameter("WkT", [D, D], F32R, isOutput=False)
    WvT = nc.declare_dram_parameter("WvT", [D, D], F32R, isOutput=False)
    idn = nc.declare_dram_parameter("idn", [128, 128], F32, isOutput=False)
    out = nc.declare_dram_parameter("out", [QL, D], F32, isOutput=True)

    xT_r = xT.rearrange("(dt p) k -> p dt k", p=128)
    xqT_r = xqT.rearrange("(dt p) q -> p dt q", p=128)
    Wq_r = WqT.rearrange("(dt p) e -> p dt e", p=128)
    Wk_r = WkT.rearrange("(dt p) e -> p dt e", p=128)
    Wv_r = WvT.rearrange("(dt p) e -> p dt e", p=128)
    out_r = out.rearrange("(qt p) e -> qt p e", p=128)

    with ExitStack() as top:
        tc = top.enter_context(tile.TileContext(nc))

        const_pool = top.enter_context(tc.tile_pool(name="const", bufs=1))
        res_pool = top.enter_context(tc.tile_pool(name="res", bufs=1))
        dram_pool = top.enter_context(
            tc.tile_pool(name="dram", bufs=1, space="DRAM")
        )

        ident = const_pool.tile([128, 128], F32)
        nc.sync.dma_start(ident[:], idn[:])

        v_sb = res_pool.tile([128, KT, D], F32R, tag="v_sb")
        kt_sb = res_pool.tile([128, ET, SEQ], F32R, tag="kt_sb")
        qts = dram_pool.tile([ET, 128, QL], F32R, tag="qts")

        # ---------------- phase Q: QT[e,q], spilled to DRAM ----------------
        with (
            tc.tile_pool(name="phq_w", bufs=1) as phq_w,
            tc.tile_pool(name="phq_s", bufs=4) as phq_s,
            tc.tile_pool(name="psq", bufs=4, space="PSUM") as psq,
        ):
            wq = phq_w.tile([128, DT, D], F32R, tag="wq")
            nc.sync.dma_start(wq[:], Wq_r[:])
            xq = phq_w.tile([128, DT, QL], F32R, tag="xq")
            nc.sync.dma_start(xq[:], xqT_r[:])
            ci = 0
            for qc in range(QL // 512):
                for et in range(ET):
                    ps = psq.tile([128, 512], F32, tag="ps")
                    for d in range(DT):
                        nc.tensor.matmul(
                            ps[:],
                            wq[:, d, et * 128 : (et + 1) * 128],
                            xq[:, d, qc * 512 : (qc + 1) * 512],
                            start=(d == 0),
                            stop=(d == DT - 1),
                        )
                    piece = phq_s.tile([128, 512], F32R, tag="piece")
                    _copy(nc, ci, piece[:], ps[:])
                    ci += 1
                    nc.sync.dma_start(
                        qts[et, :, qc * 512 : (qc + 1) * 512], piece[:]
                    )

        # ---------------- phase V: V[k,e] resident ----------------
        with (
            tc.tile_pool(name="phv_w", bufs=1) as phv_w,
            tc.tile_pool(name="phv_x", bufs=2) as phv_x,
            tc.tile_pool(name="psv", bufs=4, space="PSUM") as psv,
        ):
            wv = phv_w.tile([128, DT, D], F32R, tag="wv")
            nc.sync.dma_start(wv[:], Wv_r[:])
            ci = 0
            for j in range(NXB):
                xtb = phv_x.tile([128, DT, XB], F32R, tag="xtb")
                nc.sync.dma_start(xtb[:], xT_r[:, :, j * XB : (j + 1) * XB])
                for k2 in range(XB // 128):
                    kt = j * (XB // 128) + k2
                    for ec in range(D // 512):
                        ps = psv.tile([128, 512], F32, tag="ps")
                        for d in range(DT):
                            nc.tensor.matmul(
                                ps[:],
                                xtb[:, d, k2 * 128 : (k2 + 1) * 128],
                                wv[:, d, ec * 512 : (ec + 1) * 512],
                                start=(d == 0),
                                stop=(d == DT - 1),
                            )
                        _copy(nc, ci, v_sb[:, kt, ec * 512 : (ec + 1) * 512], ps[:])
                        ci += 1

        # ---------------- phase K: KT[e,k] resident ----------------
        with (
            tc.tile_pool(name="phk_w", bufs=1) as phk_w,
            tc.tile_pool(name="phk_x", bufs=2) as phk_x,
            tc.tile_pool(name="psk", bufs=4, space="PSUM") as psk,
        ):
            wk = phk_w.tile([128, DT, D], F32R, tag="wk")
            nc.sync.dma_start(wk[:], Wk_r[:])
            ci = 0
            for j in range(NXB):
                xtb = phk_x.tile([128, DT, XB], F32R, tag="xtb")
                nc.sync.dma_start(xtb[:], xT_r[:, :, j * XB : (j + 1) * XB])
                for et in range(ET):
                    ps = psk.tile([128, XB], F32, tag="ps")
                    for d in range(DT):
                        nc.tensor.matmul(
                            ps[:],
                            wk[:, d, et * 128 : (et + 1) * 128],
                            xtb[:, d, :],
                            start=(d == 0),
                            stop=(d == DT - 1),
                        )
                    _copy(nc, ci, kt_sb[:, et, j * XB : (j + 1) * XB], ps[:])
                    ci += 1

        # ---------------- attention ----------------
        with (
            tc.tile_pool(name="qsl_p", bufs=2) as qsl_p,
            tc.tile_pool(name="attu_p", bufs=2) as attu_p,
            tc.tile_pool(name="attt_p", bufs=4) as attt_p,
            tc.tile_pool(name="osb_p", bufs=2) as osb_p,
            tc.tile_pool(name="vec_p", bufs=4) as vec_p,
            tc.tile_pool(name="pss", bufs=1, space="PSUM") as pss,
            tc.tile_pool(name="pst", bufs=2, space="PSUM") as pst,
            tc.tile_pool(name="pso", bufs=2, space="PSUM") as pso,
        ):
            qts_r = qts[:].rearrange("et p q -> p et q")
            for qt in range(QT):
                qsl = qsl_p.tile([128, ET, 128], F32R, tag="qsl")
                nc.sync.dma_start(
                    qsl[:], qts_r[:, :, qt * 128 : (qt + 1) * 128]
                )
                S = pss.tile([128, SEQ], F32, tag="S")
                for et in range(ET):
                    for kc in range(SEQ // 512):
                        nc.tensor.matmul(
                            S[:, kc * 512 : (kc + 1) * 512],
                            qsl[:, et, :],
                            kt_sb[:, et, kc * 512 : (kc + 1) * 512],
                            start=(et == 0),
                            stop=(et == ET - 1),
                        )
                attu = attu_p.tile([128, SEQ], F32, tag="attu")
                rs = vec_p.tile([128, 1], F32, tag="rs")
                nc.scalar.activation(
                    attu[:], S[:], AF.Exp, scale=1.0 / 32.0, accum_out=rs[:]
                )
                r = vec_p.tile([128, 1], F32, tag="r")
                nc.vector.reciprocal(r[:], rs[:])

                po0 = pso.tile([128, 512], F32, tag="po")
                po1 = pso.tile([128, 512], F32, tag="po")
                pos = (po0, po1)
                for kt in range(KT):
                    tp = pst.tile([128, 128], F32, tag="tp")
                    nc.tensor.transpose(
                        tp[:], attu[:, kt * 128 : (kt + 1) * 128], ident[:]
                    )
                    at = attt_p.tile([128, 128], F32R, tag="at")
                    _copy(nc, kt, at[:], tp[:])
                    for ec in range(2):
                        nc.tensor.matmul(
                            pos[ec][:],
                            at[:],
                            v_sb[:, kt, ec * 512 : (ec + 1) * 512],
                            start=(kt == 0),
                            stop=(kt == KT - 1),
                        )
                osb = osb_p.tile([128, D], F32, tag="osb")
                for ec in range(2):
                    nc.vector.tensor_scalar_mul(
                        osb[:, ec * 512 : (ec + 1) * 512], pos[ec][:], r[:]
                    )
                nc.sync.dma_start(out_r[qt], osb[:])

    nc.compile()
    return nc


_CACHE: dict = {}


def _get_nc():
    if "nc" not in _CACHE:
        _CACHE["nc"] = build()
    return _CACHE["nc"]


def _get_runner():
    """Cached jitted shard_map executable over the 8 cores.

    Mirrors concourse.bass2jax.run_bass_via_pjrt but builds the jit once,
    so repeated kernel() calls only pay input transfer + execute.
    """
    if "runner" in _CACHE:
        return _CACHE["runner"]

    import jax
    from jax.sharding import Mesh, NamedSharding, PartitionSpec
    from jax.experimental.shard_map import shard_map

    from concourse import bass2jax, mybir as _mybir

    nc = _get_nc()
    bass2jax.install_neuronx_cc_hook()

    partition_name = (
        nc.partition_id_tensor.name if nc.partition_id_tensor else None
    )
    in_names = []
    out_names = []
    out_avals = []
    zero_outs = []
    for alloc in nc.m.functions[0].allocations:
        if not isinstance(alloc, _mybir.MemoryLocationSet):
            continue
        if alloc.kind == "ExternalInput":
            if alloc.memorylocations[0].name == partition_name:
                continue
            in_names.append(alloc.memorylocations[0].name)
        elif alloc.kind == "ExternalOutput":
            name = alloc.memorylocations[0].name
            out_names.append(name)
            shape = tuple(alloc.tensor_shape)
            dtype = _mybir.dt.np(alloc.dtype)
            out_avals.append(jax.core.ShapedArray(shape, dtype))
            zero_outs.append(np.zeros(shape, dtype))
    n_params = len(in_names)
    all_in_names = in_names + out_names
    if partition_name is not None:
        all_in_names = all_in_names + [partition_name]

    def _body(*args):
        operands = list(args)
        if partition_name is not None:
            operands.append(bass2jax.partition_id_tensor())
        outs = bass2jax._bass_exec_p.bind(
            *operands,
            out_avals=tuple(out_avals),
            in_names=tuple(all_in_names),
            out_names=tuple(out_names),
            lowering_input_output_aliases=(),
            sim_require_finite=True,
            sim_require_nnan=True,
            nc=nc,
        )
        return tuple(outs)

    devices = jax.devices()[:N_CORES]
    mesh = Mesh(np.asarray(devices), ("core",))
    spec = PartitionSpec("core")
    n_outs = len(out_names)
    # No donation: the kernel writes every element of "out", so results
    # don't need to alias the zero placeholders. This lets callers reuse
    # the same device-resident placeholder arrays across calls.
    sharded = jax.jit(
        shard_map(
            _body,
            mesh=mesh,
            in_specs=(spec,) * (n_params + n_outs),
            out_specs=(spec,) * n_outs,
            check_rep=False,
        ),
        keep_unused=True,
    )
    sharding = NamedSharding(mesh, spec)

    def run(in_maps):
        concat_in = [
            np.concatenate([np.asarray(m[name]) for m in in_maps], axis=0)
            for name in in_names
        ]
        concat_zeros = [
            np.zeros((N_CORES * z.shape[0], *z.shape[1:]), z.dtype)
            for z in zero_outs
        ]
        dev_in = [jax.device_put(a, sharding) for a in concat_in]
        dev_zero = [jax.device_put(a, sharding) for a in concat_zeros]
        out_arrs = sharded(*dev_in, *dev_zero)
        return [
            {
                name: np.asarray(out_arrs[i]).reshape(
                    N_CORES, *out_avals[i].shape
                )[c]
                for i, name in enumerate(out_names)
            }
            for c in range(N_CORES)
        ]

    def run_device(dev_in, dev_zero):
        return sharded(*dev_in, *dev_zero)

    _CACHE["runner"] = (run, run_device, sharding, in_names, zero_outs)
    return _CACHE["runner"]


def _make_in_maps(x, Wq, Wk, Wv):
    x = np.asarray(x, dtype=np.float32)
    wqT = np.ascontiguousarray(np.asarray(Wq, dtype=np.float32).T)
    wkT = np.ascontiguousarray(np.asarray(Wk, dtype=np.float32).T)
    wvT = np.ascontiguousarray(np.asarray(Wv, dtype=np.float32).T)
    eye = np.eye(128, dtype=np.float32)
    in_maps = []
    for c in range(N_CORES):
        b, h = divmod(c, 2)
        xb = np.ascontiguousarray(x[b].T)
        xq = np.ascontiguousarray(x[b, h * QL : (h + 1) * QL].T)
        in_maps.append(
            {
                "xT": xb,
                "xqT": xq,
                "WqT": wqT,
                "WkT": wkT,
                "WvT": wvT,
                "idn": eye,
            }
        )
    return in_maps


def _assemble(results):
    out = np.empty((B, SEQ, D), dtype=np.float32)
    for c in range(N_CORES):
        b, h = divmod(c, 2)
        out[b, h * QL : (h + 1) * QL] = results[c]["out"]
    return out


def run_traced(x, Wq, Wk, Wv, **kw):
    """Run via run_bass_kernel_spmd, return (output, BassKernelResults)."""
    nc = _get_nc()
    res = run_bass_kernel_spmd(
        nc, _make_in_maps(x, Wq, Wk, Wv), list(range(N_CORES)), **kw
    )
    return _assemble(res.results), res


def kernel(x, Wq, Wk, Wv):
    run, _, _, _, _ = _get_runner()
    results = run(_make_in_maps(x, Wq, Wk, Wv))
    return _assemble(results)


# revision 9
# speedup vs baseline: 11640.7085x; 160.9055x over previous
"""Trainium2 Bass kernel for single-head base attention.

Problem: x [4, 2048, 1024] fp32; Wq/Wk/Wv [1024, 1024] (torch [out, in]).
  Q = x @ Wq.T ; K = x @ Wk.T ; V = x @ Wv.T
  out = softmax(Q K^T / 32) V

Sharding: 8 cores = 4 batches x 2 query-halves. Each core computes K/V for
its batch's full 2048-seq and Q for its 1024-query half; outputs are
disjoint [1024, 1024] slices, so no collectives.

Per-core schedule (all matmuls float32r = tf32-rate, fp32 accumulate):
  phase Q: QT[e,q] = WqT.T @ xqT     -> spilled to DRAM scratch
  phase V: V[k,e]  = xT.T @ WvT      -> resident SBUF (8MB)
  phase K: KT[e,k] = WkT.T @ xT      -> resident SBUF (8MB)
  attention per 128-query tile:
      S[q,k] = QT_sl.T @ KT          (PSUM, 4 banks)
      attU = exp(S/32) (ACT, accum_out = row sums; no max-subtraction
             needed: |S/32| <= ~6 so exp is well within fp32 range)
      attT = PE-transpose(attU) 128x128 blocks
      O[q,e] = attT.T @ V            (PSUM accumulate over k)
      out = O * (1/rowsum)           -> DRAM
"""

import os
import sys
from contextlib import ExitStack

import numpy as np

for _p in ("/opt/trn_rl_repo", "/root/.axon_site/_ro/trn_rl_repo"):
    if os.path.isdir(_p) and _p not in sys.path:
        sys.path.append(_p)

import concourse.bass as bass
import concourse.mybir as mybir
from concourse import bacc, tile
from concourse.bass_utils import run_bass_kernel_spmd

F32 = mybir.dt.float32
F32R = mybir.dt.float32r  # tf32-rate matmul dtype, fp32 storage bits

B, SEQ, D = 4, 2048, 1024
QL = SEQ // 2          # queries per core
N_CORES = 8
DT = D // 128          # 8 d-tiles (contraction)
ET = D // 128          # 8 e-tiles (hidden out)
KT = SEQ // 128        # 16 k-tiles
QT = QL // 128         # 8 q-tiles per core
XB = 256               # xT streaming col-block width
NXB = SEQ // XB        # 8 blocks
AF = mybir.ActivationFunctionType


def _copy(nc, i, dst, src):
    # alternate PSUM->SBUF copies between DVE and ACT to balance engines
    if i % 2 == 0:
        nc.vector.tensor_copy(dst, src)
    else:
        nc.scalar.copy(dst, src)


def build(reps: int = 1):
    nc = bacc.Bacc(
        "TRN2", target_bir_lowering=False, debug=False, num_devices=N_CORES
    )

    xT = nc.declare_dram_parameter("xT", [D, SEQ], F32R, isOutput=False)
    xqT = nc.declare_dram_parameter("xqT", [D, QL], F32R, isOutput=False)
    WqT = nc.declare_dram_parameter("WqT", [D, D], F32R, isOutput=False)
    WkT = nc.declare_dram_parameter("WkT", [D, D], F32R, isOutput=False)
    WvT = nc.declare_dram_parameter("WvT", [D, D], F32R, isOutput=False)
    idn = nc.declare_dram_parameter("idn", [128, 128], F32, isOutput=False)
    out = nc.declare_dram_parameter("out", [QL, D], F32, isOutput=True)

    xT_r = xT.rearrange("(dt p) k -> p dt k", p=128)
    xqT_r = xqT.rearrange("(dt p) q -> p dt q", p=128)
    Wq_r = WqT.rearrange("(dt p) e -> p dt e", p=128)
    Wk_r = WkT.rearrange("(dt p) e -> p dt e", p=128)
    Wv_r = WvT.rearrange("(dt p) e -> p dt e", p=128)
    out_r = out.rearrange("(qt p) e -> qt p e", p=128)

    with ExitStack() as top:
        tc = top.enter_context(tile.TileContext(nc))

        const_pool = top.enter_context(tc.tile_pool(name="const", bufs=1))
        res_pool = top.enter_context(tc.tile_pool(name="res", bufs=1))
        dram_pool = top.enter_context(
            tc.tile_pool(name="dram", bufs=1, space="DRAM")
        )

        ident = const_pool.tile([128, 128], F32)
        nc.sync.dma_start(ident[:], idn[:])

        v_sb = res_pool.tile([128, KT, D], F32R, tag="v_sb")
        kt_sb = res_pool.tile([128, ET, SEQ], F32R, tag="kt_sb")
        qts = dram_pool.tile([ET, 128, QL], F32R, tag="qts")

        for _rep in range(reps):
            _build_body(nc, tc, ident, v_sb, kt_sb, qts, locals())

    nc.compile()
    return nc


def _build_body(nc, tc, ident, v_sb, kt_sb, qts, env):
    xT_r = env["xT_r"]
    xqT_r = env["xqT_r"]
    Wq_r = env["Wq_r"]
    Wk_r = env["Wk_r"]
    Wv_r = env["Wv_r"]
    out_r = env["out_r"]
    if True:
        # ---------------- phase Q: QT[e,q], spilled to DRAM ----------------
        with (
            tc.tile_pool(name="phq_w", bufs=1) as phq_w,
            tc.tile_pool(name="phq_s", bufs=4) as phq_s,
            tc.tile_pool(name="psq", bufs=4, space="PSUM") as psq,
        ):
            wq = phq_w.tile([128, DT, D], F32R, tag="wq")
            nc.sync.dma_start(wq[:], Wq_r[:])
            xq = phq_w.tile([128, DT, QL], F32R, tag="xq")
            nc.sync.dma_start(xq[:], xqT_r[:])
            ci = 0
            for qc in range(QL // 512):
                for et in range(ET):
                    ps = psq.tile([128, 512], F32, tag="ps")
                    for d in range(DT):
                        nc.tensor.matmul(
                            ps[:],
                            wq[:, d, et * 128 : (et + 1) * 128],
                            xq[:, d, qc * 512 : (qc + 1) * 512],
                            start=(d == 0),
                            stop=(d == DT - 1),
                        )
                    piece = phq_s.tile([128, 512], F32R, tag="piece")
                    _copy(nc, ci, piece[:], ps[:])
                    ci += 1
                    nc.sync.dma_start(
                        qts[et, :, qc * 512 : (qc + 1) * 512], piece[:]
                    )

        # ---------------- phase V: V[k,e] resident ----------------
        with (
            tc.tile_pool(name="phv_w", bufs=1) as phv_w,
            tc.tile_pool(name="phv_x", bufs=2) as phv_x,
            tc.tile_pool(name="psv", bufs=4, space="PSUM") as psv,
        ):
            wv = phv_w.tile([128, DT, D], F32R, tag="wv")
            nc.sync.dma_start(wv[:], Wv_r[:])
            ci = 0
            for j in range(NXB):
                xtb = phv_x.tile([128, DT, XB], F32R, tag="xtb")
                nc.sync.dma_start(xtb[:], xT_r[:, :, j * XB : (j + 1) * XB])
                for k2 in range(XB // 128):
                    kt = j * (XB // 128) + k2
                    for ec in range(D // 512):
                        ps = psv.tile([128, 512], F32, tag="ps")
                        for d in range(DT):
                            nc.tensor.matmul(
                                ps[:],
                                xtb[:, d, k2 * 128 : (k2 + 1) * 128],
                                wv[:, d, ec * 512 : (ec + 1) * 512],
                                start=(d == 0),
                                stop=(d == DT - 1),
                            )
                        _copy(nc, ci, v_sb[:, kt, ec * 512 : (ec + 1) * 512], ps[:])
                        ci += 1

        # ---------------- phase K: KT[e,k] resident ----------------
        with (
            tc.tile_pool(name="phk_w", bufs=1) as phk_w,
            tc.tile_pool(name="phk_x", bufs=2) as phk_x,
            tc.tile_pool(name="psk", bufs=4, space="PSUM") as psk,
        ):
            wk = phk_w.tile([128, DT, D], F32R, tag="wk")
            nc.sync.dma_start(wk[:], Wk_r[:])
            ci = 0
            for j in range(NXB):
                xtb = phk_x.tile([128, DT, XB], F32R, tag="xtb")
                nc.sync.dma_start(xtb[:], xT_r[:, :, j * XB : (j + 1) * XB])
                for et in range(ET):
                    ps = psk.tile([128, XB], F32, tag="ps")
                    for d in range(DT):
                        nc.tensor.matmul(
                            ps[:],
                            wk[:, d, et * 128 : (et + 1) * 128],
                            xtb[:, d, :],
                            start=(d == 0),
                            stop=(d == DT - 1),
                        )
                    _copy(nc, ci, kt_sb[:, et, j * XB : (j + 1) * XB], ps[:])
                    ci += 1

        # ---------------- attention ----------------
        with (
            tc.tile_pool(name="qsl_p", bufs=2) as qsl_p,
            tc.tile_pool(name="attu_p", bufs=2) as attu_p,
            tc.tile_pool(name="attt_p", bufs=4) as attt_p,
            tc.tile_pool(name="osb_p", bufs=2) as osb_p,
            tc.tile_pool(name="vec_p", bufs=4) as vec_p,
            tc.tile_pool(name="pss", bufs=1, space="PSUM") as pss,
            tc.tile_pool(name="pst", bufs=2, space="PSUM") as pst,
            tc.tile_pool(name="pso", bufs=2, space="PSUM") as pso,
        ):
            qts_r = qts[:].rearrange("et p q -> p et q")
            for qt in range(QT):
                qsl = qsl_p.tile([128, ET, 128], F32R, tag="qsl")
                nc.sync.dma_start(
                    qsl[:], qts_r[:, :, qt * 128 : (qt + 1) * 128]
                )
                S = pss.tile([128, SEQ], F32, tag="S")
                for et in range(ET):
                    for kc in range(SEQ // 512):
                        nc.tensor.matmul(
                            S[:, kc * 512 : (kc + 1) * 512],
                            qsl[:, et, :],
                            kt_sb[:, et, kc * 512 : (kc + 1) * 512],
                            start=(et == 0),
                            stop=(et == ET - 1),
                        )
                attu = attu_p.tile([128, SEQ], F32, tag="attu")
                rs = vec_p.tile([128, 1], F32, tag="rs")
                nc.scalar.activation(
                    attu[:], S[:], AF.Exp, scale=1.0 / 32.0, accum_out=rs[:]
                )
                r = vec_p.tile([128, 1], F32, tag="r")
                nc.vector.reciprocal(r[:], rs[:])

                po0 = pso.tile([128, 512], F32, tag="po")
                po1 = pso.tile([128, 512], F32, tag="po")
                pos = (po0, po1)
                for kt in range(KT):
                    tp = pst.tile([128, 128], F32, tag="tp")
                    nc.tensor.transpose(
                        tp[:], attu[:, kt * 128 : (kt + 1) * 128], ident[:]
                    )
                    at = attt_p.tile([128, 128], F32R, tag="at")
                    _copy(nc, kt, at[:], tp[:])
                    for ec in range(2):
                        nc.tensor.matmul(
                            pos[ec][:],
                            at[:],
                            v_sb[:, kt, ec * 512 : (ec + 1) * 512],
                            start=(kt == 0),
                            stop=(kt == KT - 1),
                        )
                osb = osb_p.tile([128, D], F32, tag="osb")
                for ec in range(2):
                    nc.vector.tensor_scalar_mul(
                        osb[:, ec * 512 : (ec + 1) * 512], pos[ec][:], r[:]
                    )
                nc.sync.dma_start(out_r[qt], osb[:])


_CACHE: dict = {}


def _get_nc():
    if "nc" not in _CACHE:
        _CACHE["nc"] = build()
    return _CACHE["nc"]


def _get_runner():
    """Cached jitted shard_map executable over the 8 cores.

    Mirrors concourse.bass2jax.run_bass_via_pjrt but builds the jit once,
    so repeated kernel() calls only pay input transfer + execute.
    """
    if "runner" in _CACHE:
        return _CACHE["runner"]

    import jax
    from jax.sharding import Mesh, NamedSharding, PartitionSpec
    from jax.experimental.shard_map import shard_map

    from concourse import bass2jax, mybir as _mybir

    nc = _get_nc()
    bass2jax.install_neuronx_cc_hook()

    partition_name = (
        nc.partition_id_tensor.name if nc.partition_id_tensor else None
    )
    in_names = []
    out_names = []
    out_avals = []
    zero_outs = []
    for alloc in nc.m.functions[0].allocations:
        if not isinstance(alloc, _mybir.MemoryLocationSet):
            continue
        if alloc.kind == "ExternalInput":
            if alloc.memorylocations[0].name == partition_name:
                continue
            in_names.append(alloc.memorylocations[0].name)
        elif alloc.kind == "ExternalOutput":
            name = alloc.memorylocations[0].name
            out_names.append(name)
            shape = tuple(alloc.tensor_shape)
            dtype = _mybir.dt.np(alloc.dtype)
            out_avals.append(jax.core.ShapedArray(shape, dtype))
            zero_outs.append(np.zeros(shape, dtype))
    n_params = len(in_names)
    all_in_names = in_names + out_names
    if partition_name is not None:
        all_in_names = all_in_names + [partition_name]

    def _body(*args):
        operands = list(args)
        if partition_name is not None:
            operands.append(bass2jax.partition_id_tensor())
        outs = bass2jax._bass_exec_p.bind(
            *operands,
            out_avals=tuple(out_avals),
            in_names=tuple(all_in_names),
            out_names=tuple(out_names),
            lowering_input_output_aliases=(),
            sim_require_finite=True,
            sim_require_nnan=True,
            nc=nc,
        )
        return tuple(outs)

    devices = jax.devices()[:N_CORES]
    mesh = Mesh(np.asarray(devices), ("core",))
    spec = PartitionSpec("core")
    n_outs = len(out_names)
    # No donation: the kernel writes every element of "out", so results
    # don't need to alias the zero placeholders. This lets callers reuse
    # the same device-resident placeholder arrays across calls.
    sharded = jax.jit(
        shard_map(
            _body,
            mesh=mesh,
            in_specs=(spec,) * (n_params + n_outs),
            out_specs=(spec,) * n_outs,
            check_rep=False,
        ),
        keep_unused=True,
    )
    sharding = NamedSharding(mesh, spec)

    def run(in_maps):
        concat_in = [
            np.concatenate([np.asarray(m[name]) for m in in_maps], axis=0)
            for name in in_names
        ]
        concat_zeros = [
            np.zeros((N_CORES * z.shape[0], *z.shape[1:]), z.dtype)
            for z in zero_outs
        ]
        dev_in = [jax.device_put(a, sharding) for a in concat_in]
        dev_zero = [jax.device_put(a, sharding) for a in concat_zeros]
        out_arrs = sharded(*dev_in, *dev_zero)
        return [
            {
                name: np.asarray(out_arrs[i]).reshape(
                    N_CORES, *out_avals[i].shape
                )[c]
                for i, name in enumerate(out_names)
            }
            for c in range(N_CORES)
        ]

    def run_device(dev_in, dev_zero):
        return sharded(*dev_in, *dev_zero)

    _CACHE["runner"] = (run, run_device, sharding, in_names, zero_outs)
    return _CACHE["runner"]


def _make_in_maps(x, Wq, Wk, Wv):
    x = np.asarray(x, dtype=np.float32)
    wqT = np.ascontiguousarray(np.asarray(Wq, dtype=np.float32).T)
    wkT = np.ascontiguousarray(np.asarray(Wk, dtype=np.float32).T)
    wvT = np.ascontiguousarray(np.asarray(Wv, dtype=np.float32).T)
    eye = np.eye(128, dtype=np.float32)
    in_maps = []
    for c in range(N_CORES):
        b, h = divmod(c, 2)
        xb = np.ascontiguousarray(x[b].T)
        xq = np.ascontiguousarray(x[b, h * QL : (h + 1) * QL].T)
        in_maps.append(
            {
                "xT": xb,
                "xqT": xq,
                "WqT": wqT,
                "WkT": wkT,
                "WvT": wvT,
                "idn": eye,
            }
        )
    return in_maps


def _assemble(results):
    out = np.empty((B, SEQ, D), dtype=np.float32)
    for c in range(N_CORES):
        b, h = divmod(c, 2)
        out[b, h * QL : (h + 1) * QL] = results[c]["out"]
    return out


def run_traced(x, Wq, Wk, Wv, **kw):
    """Run via run_bass_kernel_spmd, return (output, BassKernelResults)."""
    nc = _get_nc()
    res = run_bass_kernel_spmd(
        nc, _make_in_maps(x, Wq, Wk, Wv), list(range(N_CORES)), **kw
    )
    return _assemble(res.results), res


def kernel(x, Wq, Wk, Wv):
    run, _, _, _, _ = _get_runner()
    results = run(_make_in_maps(x, Wq, Wk, Wv))
    return _assemble(results)
